# revision 1
# baseline (speedup 1.0000x reference)
"""GPNNCell (gnn_message_passing) Trainium2 Bass kernel.

Full-input contract: kernel(**inputs) takes the complete tensors from
setup_inputs() and returns the full [8, 64, 64->sum, 768] output, i.e.
node_features + sum_w weight_edge * merged_message   -> [8, 64, 768].

Distribution: data-parallel over batch B=8, one batch element per NeuronCore,
no collectives. Per core the whole cell is computed as a chain of f32r
(TF32-like, 1 cyc/row) matmuls on the tensor engine:

  edge rows are processed source-node(w)-major in 8 blocks of 512 rows
  (8 w x 64 v). Per block:
    X^T[feat, row]   via PE transpose of DMA'd edge tiles
    gates^T          = Wg[i|g|o].T @ X^T          (f-gate skipped: c0 = 0)
    h^T              = sig(o)*tanh(sig(i)*tanh(g))     (ACT + DVE, bf16)
    w_edge           = sigmoid(W_lout.T @ h^T)    (bf16 matmul, M=1)
    msg^T            = Wmsg_bot.T @ X^T + P^T[w]  (P^T = Wmsg_top.T@node^T+b,
                                                   broadcast over v via 0-step AP)
    m[row, feat]     = msg^T_tile.T @ W_mrg       (layout flip: rows on partitions)
    LayerNorm        bn_stats/bn_aggr, 1/sqrt(var+eps), fused tensor_scalar
    GELU (erf)       ACT
    wm               = w_edge * gelu   (bf16)
    acc[v, feat]    += I2stack.T @ wm             (sum over w, psum-resident
                                                   across the whole kernel)
  out = node + acc.
"""
import numpy as np
import ml_dtypes
from contextlib import ExitStack

import concourse.mybir as mybir
import concourse.tile as tile
from concourse import bacc
from concourse.bass_utils import run_bass_kernel_spmd
from concourse.masks import make_identity

F32 = mybir.dt.float32
F32R = mybir.dt.float32r
BF16 = mybir.dt.bfloat16
AF = mybir.ActivationFunctionType
OP = mybir.AluOpType

B = 8           # batch == number of cores
N = 64          # nodes
D = 768         # feature dim
H = 256         # lstm hidden
ROWS = N * N    # 4096 edge rows per core
BLK = 512       # rows per block (8 w x 64 v)
NBLK = ROWS // BLK
TPB = BLK // 128
KD = D // 128
LN_EPS = 1e-12


def build(apply_bmrg=True, apply_lng=True, apply_lnb=True, reps=1):
    nc = bacc.Bacc(None)

    edge = nc.dram_tensor("edge", (ROWS, D), F32, kind="ExternalInput")
    node = nc.dram_tensor("node", (N, D), F32, kind="ExternalInput")
    Wg = nc.dram_tensor("W_gates", (D, 4 * H), F32, kind="ExternalInput")
    bg = nc.dram_tensor("b_gates", (4 * H,), F32, kind="ExternalInput")
    Wl = nc.dram_tensor("W_lout", (H, 1), F32, kind="ExternalInput")
    bl = nc.dram_tensor("b_lout", (1,), F32, kind="ExternalInput")
    Wm = nc.dram_tensor("W_msg", (2 * D, D), F32, kind="ExternalInput")
    bm = nc.dram_tensor("b_msg", (D,), F32, kind="ExternalInput")
    Wr = nc.dram_tensor("W_mrg", (D, D), F32, kind="ExternalInput")
    br = nc.dram_tensor("b_mrg", (D,), F32, kind="ExternalInput")
    lg = nc.dram_tensor("ln_g", (D,), F32, kind="ExternalInput")
    lb = nc.dram_tensor("ln_b", (D,), F32, kind="ExternalInput")
    out = nc.dram_tensor("out", (N, D), F32, kind="ExternalOutput")

    # stacked identity [128, 64] bf16: partitions (w_local 2, v 64) -> v
    i2_np = np.tile(np.eye(N, dtype=np.float32), (2, 1)).astype(ml_dtypes.bfloat16)
    i2_dram = nc.inline_tensor(i2_np, name="i2_stack")
    onesrow_dram = nc.inline_tensor(np.ones((1, 128), np.float32), name="ones_row")

    with tile.TileContext(nc) as tc, ExitStack() as ctx:
        W = ctx.enter_context(tc.tile_pool(name="W", bufs=1))          # persistent
        xnp = ctx.enter_context(tc.tile_pool(name="xn", bufs=3))
        xtp = ctx.enter_context(tc.tile_pool(name="xt", bufs=2))
        hp = ctx.enter_context(tc.tile_pool(name="h", bufs=2))
        msgp = ctx.enter_context(tc.tile_pool(name="msg", bufs=2))
        tmp = ctx.enter_context(tc.tile_pool(name="tmp", bufs=4))
        lnp = ctx.enter_context(tc.tile_pool(name="ln", bufs=4))
        wmp = ctx.enter_context(tc.tile_pool(name="wm", bufs=2))
        sml = ctx.enter_context(tc.tile_pool(name="sml", bufs=4))
        drp = ctx.enter_context(tc.tile_pool(name="dr", bufs=2, space="DRAM"))

        ps1 = ctx.enter_context(tc.tile_pool(name="ps1", bufs=2, space="PSUM"))
        psm = ctx.enter_context(tc.tile_pool(name="psm", bufs=3, space="PSUM"))
        psf = ctx.enter_context(tc.tile_pool(name="psf", bufs=1, space="PSUM"))
        pst = ctx.enter_context(tc.tile_pool(name="pst", bufs=1, space="PSUM"))

        # ---------------- persistent weights / constants ----------------
        ident = W.tile([128, 128], F32, tag="ident")
        make_identity(nc, ident[:])

        # W_gates cols: i=[0:256], g=[512:768], o=[768:1024] -> packed [i|g|o]
        wg_sb = W.tile([128, KD, D], F32R, tag="wg")
        for k in range(KD):
            for j, (lo, hi) in enumerate([(0, 256), (512, 768), (768, 1024)]):
                nc.gpsimd.dma_start(wg_sb[:, k, j * 256:(j + 1) * 256],
                                    Wg[k * 128:(k + 1) * 128, lo:hi])
        wmt_sb = W.tile([128, KD, D], F32R, tag="wmt")
        wmb_sb = W.tile([128, KD, D], F32R, tag="wmb")
        wmg_sb = W.tile([128, KD, D], F32R, tag="wmg")
        for k in range(KD):
            nc.gpsimd.dma_start(wmt_sb[:, k, :], Wm[k * 128:(k + 1) * 128, :])
            nc.gpsimd.dma_start(wmb_sb[:, k, :], Wm[D + k * 128:D + (k + 1) * 128, :])
            nc.gpsimd.dma_start(wmg_sb[:, k, :], Wr[k * 128:(k + 1) * 128, :])
        wl_sb = W.tile([128, 2, 1], BF16, tag="wl")
        nc.gpsimd.dma_start(wl_sb[:, 0, :], Wl[0:128, :])
        nc.gpsimd.dma_start(wl_sb[:, 1, :], Wl[128:256, :])

        # biases: b_gates [1024] -> [128, 8]; chunk cols i0=0 i1=1 g0=4 g1=5 o0=6 o1=7
        bg_sb = W.tile([128, 8], F32, tag="bg")
        nc.sync.dma_start(bg_sb[:], bg[:].rearrange("(c p) -> p c", p=128))
        bm_sb = W.tile([128, KD], F32, tag="bm")
        nc.sync.dma_start(bm_sb[:], bm[:].rearrange("(c p) -> p c", p=128))
        bl_sb = W.tile([128, 1], F32, tag="bl")
        nc.sync.dma_start(bl_sb[:], bl[:].partition_broadcast(128))

        gfull = W.tile([128, D], F32, tag="gfull")
        nc.sync.dma_start(gfull[:], lg[:].partition_broadcast(128))
        bfull = W.tile([128, D], F32, tag="bfull")
        nc.sync.dma_start(bfull[:], lb[:].partition_broadcast(128))

        i2_sb = W.tile([128, N], BF16, tag="i2")
        nc.sync.dma_start(i2_sb[:], i2_dram[:])
        onesrow_f = W.tile([1, 128], F32, tag="onesrowf")
        nc.sync.dma_start(onesrow_f[:], onesrow_dram[:])
        onesrow = W.tile([1, 128], F32R, tag="onesrow")
        nc.vector.tensor_copy(onesrow[:], onesrow_f[:])
        brrow = W.tile([1, D], F32R, tag="brrow")
        nc.gpsimd.dma_start(brrow[:], br[:].rearrange("(a c) -> a c", a=1))

        eps_sb = W.tile([128, 1], F32, tag="eps")
        nc.gpsimd.memset(eps_sb[:], LN_EPS)

        node_sb = W.tile([N, D], F32, tag="node")
        nc.sync.dma_start(node_sb[:], node[:])

        # node^T [128, KD, 64] f32r via PE transpose
        node_t = W.tile([128, KD, N], F32R, tag="nodet")
        for k in range(KD):
            ptt = pst.tile([128, 128], F32, tag="tp")
            nc.tensor.transpose(ptt[:, 0:N], node_sb[0:N, k * 128:(k + 1) * 128],
                                ident[0:N, 0:N])
            nc.vector.tensor_copy(node_t[:, k, :], ptt[:, 0:N])

        # P^T [128, KD, 64] = Wmsg_top.T @ node^T (+ b_msg)
        p_sb = W.tile([128, KD, N], F32, tag="p")
        for m in range(KD):
            pp = ps1.tile([128, 512], F32, tag="s1")
            for k in range(KD):
                nc.tensor.matmul(pp[:, 0:N], wmt_sb[:, k, m * 128:(m + 1) * 128],
                                 node_t[:, k, :], start=(k == 0), stop=(k == KD - 1))
            nc.vector.tensor_scalar(p_sb[:, m, :], pp[:, 0:N], bm_sb[:, m:m + 1],
                                    None, OP.add)

        # final accumulators [64, 384] x2 (psum, live across main loop)
        acc_lo = psf.tile([N, 384], F32, tag="acclo")
        acc_hi = psf.tile([N, 384], F32, tag="acchi")

        out_sb = W.tile([N, D], F32, tag="out")

        # ---------------- main loop (body repeated `reps` times for timing) ----
        def body():
            for blk in range(NBLK):
                # 1. load (w-major: tile t covers w = blk*8+2t, +1) + 2. transpose
                e3 = edge[:].rearrange("(v w) d -> w v d", w=N)
                xt = xtp.tile([128, KD, BLK], F32R, tag="xt")
                for t in range(TPB):
                    w0 = blk * 8 + 2 * t
                    xn = xnp.tile([128, D], F32, tag="xn")
                    nc.sync.dma_start(xn[0:N, :], e3[w0])
                    nc.sync.dma_start(xn[N:128, :], e3[w0 + 1])
                    for k in range(KD):
                        ptt = pst.tile([128, 128], F32, tag="tp")
                        nc.tensor.transpose(ptt[:], xn[:, k * 128:(k + 1) * 128], ident[:])
                        dst = xt[:, k, t * 128:(t + 1) * 128]
                        if (t * KD + k) % 2 == 0:
                            nc.vector.tensor_copy(dst, ptt[:])
                        else:
                            nc.scalar.activation(dst, ptt[:], AF.Identity)

                # 3. gates (order i,g,o per half; bias cols 0,4,1,5,6,7)
                def gate_mm(mchunk):
                    pg = ps1.tile([128, BLK], F32, tag="s1")
                    for k in range(KD):
                        nc.tensor.matmul(pg[:], wg_sb[:, k, mchunk * 128:(mchunk + 1) * 128],
                                         xt[:, k, :], start=(k == 0), stop=(k == KD - 1))
                    return pg

                h_sb = hp.tile([128, 2, BLK], BF16, tag="h")
                for half in range(2):
                    pg_i = gate_mm(half)
                    sig_i = tmp.tile([128, BLK], F32, tag="tmp")
                    nc.scalar.activation(sig_i[:], pg_i[:], AF.Sigmoid,
                                         bias=bg_sb[:, half:half + 1])
                    pg_g = gate_mm(2 + half)
                    tan_g = tmp.tile([128, BLK], F32, tag="tmp")
                    nc.scalar.activation(tan_g[:], pg_g[:], AF.Tanh,
                                         bias=bg_sb[:, 4 + half:5 + half])
                    c_t = tmp.tile([128, BLK], F32, tag="tmp")
                    nc.vector.tensor_tensor(c_t[:], sig_i[:], tan_g[:], OP.mult)
                    tan_c = tmp.tile([128, BLK], F32, tag="tmp")
                    nc.scalar.activation(tan_c[:], c_t[:], AF.Tanh)
                    pg_o = gate_mm(4 + half)
                    sig_o = tmp.tile([128, BLK], F32, tag="tmp")
                    nc.scalar.activation(sig_o[:], pg_o[:], AF.Sigmoid,
                                         bias=bg_sb[:, 6 + half:7 + half])
                    nc.vector.tensor_tensor(h_sb[:, half, :], sig_o[:], tan_c[:], OP.mult)

                # 4. edge weight -> wt [128 rows, TPB] via DRAM bounce reshape
                pw = ps1.tile([1, BLK], F32, tag="s1")
                for k in range(2):
                    nc.tensor.matmul(pw[:], wl_sb[:, k, :], h_sb[:, k, :],
                                     start=(k == 0), stop=(k == 1))
                wrow = sml.tile([1, BLK], F32, tag="wrow")
                nc.vector.tensor_copy(wrow[:], pw[:])
                wdr = drp.tile([1, BLK], F32, tag="wdr")
                nc.sync.dma_start(wdr[:], wrow[:])
                wt_pre = sml.tile([128, TPB], F32, tag="wtpre")
                nc.sync.dma_start(wt_pre[:],
                                  wdr[0:1, :].rearrange("a (t p) -> (a p) t", p=128))
                wt = sml.tile([128, TPB], F32, tag="wt")
                nc.scalar.activation(wt[:], wt_pre[:], AF.Sigmoid, bias=bl_sb[:])

                # 5. message  msg^T = Wmsg_bot.T @ X^T + P^T[w] (bcast over v)
                msg = msgp.tile([128, KD, BLK], F32R, tag="msg")
                for m in range(KD):
                    pmb = ps1.tile([128, BLK], F32, tag="s1")
                    for k in range(KD):
                        nc.tensor.matmul(pmb[:], wmb_sb[:, k, m * 128:(m + 1) * 128],
                                         xt[:, k, :], start=(k == 0), stop=(k == KD - 1))
                    nc.vector.tensor_tensor(
                        msg[:, m, :].rearrange("p (w v) -> p w v", w=8),
                        pmb[:].rearrange("p (w v) -> p w v", w=8),
                        p_sb[:, m, blk * 8:blk * 8 + 8][:, :, None]
                            .broadcast_to((128, 8, N)),
                        OP.add)

                # 6-8. merge + LN + gelu + weighted reduce, per row-tile
                for t in range(TPB):
                    mlo = psm.tile([128, 384], F32, tag="pm")
                    mhi = psm.tile([128, 384], F32, tag="pm")
                    for k in range(KD):
                        lhs = msg[:, k, t * 128:(t + 1) * 128]
                        nc.tensor.matmul(mlo[:], lhs, wmg_sb[:, k, 0:384],
                                         start=(k == 0),
                                         stop=(k == KD - 1) and not apply_bmrg)
                        nc.tensor.matmul(mhi[:], lhs, wmg_sb[:, k, 384:768],
                                         start=(k == 0),
                                         stop=(k == KD - 1) and not apply_bmrg)
                    if apply_bmrg:
                        nc.tensor.matmul(mlo[:], onesrow[:], brrow[0:1, 0:384],
                                         start=False, stop=True)
                        nc.tensor.matmul(mhi[:], onesrow[:], brrow[0:1, 384:768],
                                         start=False, stop=True)

                    stats = sml.tile([128, 2, 6], F32, tag="stats")
                    nc.vector.bn_stats(stats[:, 0, :], mlo[:])
                    nc.vector.bn_stats(stats[:, 1, :], mhi[:])
                    mv = sml.tile([128, 2], F32, tag="mv")
                    nc.vector.bn_aggr(mv[:], stats[:])
                    sd = sml.tile([128, 1], F32, tag="sd")
                    nc.scalar.activation(sd[:], mv[:, 1:2], AF.Sqrt, bias=eps_sb[:])
                    istd = sml.tile([128, 1], F32, tag="istd")
                    nc.vector.reciprocal(istd[:], sd[:])

                    wm = wmp.tile([128, 2, 384], BF16, tag="wm")
                    for hf, mps in ((0, mlo), (1, mhi)):
                        y = lnp.tile([128, 384], F32, tag="y")
                        nc.vector.tensor_scalar(y[:], mps[:], mv[:, 0:1], istd[:],
                                                OP.subtract, OP.mult)
                        if apply_lng:
                            z = lnp.tile([128, 384], F32, tag="y")
                            nc.vector.tensor_tensor(
                                z[:], y[:], gfull[:, hf * 384:(hf + 1) * 384], OP.mult)
                            y = z
                        if apply_lnb:
                            z = lnp.tile([128, 384], F32, tag="y")
                            nc.vector.tensor_tensor(
                                z[:], y[:], bfull[:, hf * 384:(hf + 1) * 384], OP.add)
                            y = z
                        gl = lnp.tile([128, 384], F32, tag="y")
                        nc.scalar.activation(gl[:], y[:], AF.Gelu)
                        nc.vector.tensor_scalar(wm[:, hf, :], gl[:], wt[:, t:t + 1],
                                                None, OP.mult)

                    first = blk == 0 and t == 0
                    last = blk == NBLK - 1 and t == TPB - 1
                    nc.tensor.matmul(acc_lo[:], i2_sb[:], wm[:, 0, :],
                                     start=first, stop=last, skip_group_check=True)
                    nc.tensor.matmul(acc_hi[:], i2_sb[:], wm[:, 1, :],
                                     start=first, stop=last, skip_group_check=True)

            # 9. residual + store
            nc.vector.scalar_tensor_tensor(out_sb[:, 0:384], acc_lo[:], 0.0,
                                           node_sb[:, 0:384], OP.add, OP.add)
            nc.vector.scalar_tensor_tensor(out_sb[:, 384:768], acc_hi[:], 0.0,
                                           node_sb[:, 384:768], OP.add, OP.add)
            nc.sync.dma_start(out[:], out_sb[:])

        if reps == 1:
            body()
        else:
            with tc.For_i(0, reps, 1):
                body()

    nc.finalize()
    return nc


_CACHE = {}


def _get_nc(flags, reps=1):
    key = (flags, reps)
    if key not in _CACHE:
        _CACHE[key] = build(apply_bmrg=flags[0], apply_lng=flags[1],
                            apply_lnb=flags[2], reps=reps)
    return _CACHE[key]


def _flags(inputs):
    return (bool(np.any(inputs["b_mrg"])),
            not bool(np.allclose(inputs["ln_g"], 1.0)),
            bool(np.any(inputs["ln_b"])))


def _in_maps(inputs):
    e = np.ascontiguousarray(inputs["edge_features"], np.float32).reshape(B, ROWS, D)
    nf = np.ascontiguousarray(inputs["node_features"], np.float32)
    wkeys = ["W_gates", "b_gates", "W_lout", "b_lout", "W_msg", "b_msg",
             "W_mrg", "b_mrg", "ln_g", "ln_b"]
    w = {k: np.ascontiguousarray(inputs[k], np.float32) for k in wkeys}
    return [dict(edge=e[b], node=nf[b], **w) for b in range(B)]


def kernel(**inputs):
    nc = _get_nc(_flags(inputs))
    res = run_bass_kernel_spmd(nc, _in_maps(inputs), list(range(B)))
    return np.stack([res.results[b]["out"] for b in range(B)]).astype(np.float32)


def run_timed(inputs, reps):
    """Run the reps-looped variant once; returns (output, wall_seconds)."""
    import time
    nc = _get_nc(_flags(inputs), reps=reps)
    maps = _in_maps(inputs)
    t0 = time.time()
    res = run_bass_kernel_spmd(nc, maps, list(range(B)))
    dt = time.time() - t0
    out = np.stack([res.results[b]["out"] for b in range(B)]).astype(np.float32)
    return out, dt


# revision 11
# speedup vs baseline: 2.8580x; 2.8580x over previous
"""GPNNCell (gnn_message_passing) Trainium2 Bass kernel.

Full-input contract: kernel(**inputs) takes the complete tensors from
setup_inputs() and returns the full [8, 64, 64->sum, 768] output, i.e.
node_features + sum_w weight_edge * merged_message   -> [8, 64, 768].

Distribution: data-parallel over batch B=8, one batch element per NeuronCore,
no collectives. Per core the whole cell is computed as a chain of f32r
(TF32-like, 1 cyc/row) matmuls on the tensor engine:

  edge rows are processed source-node(w)-major in 8 blocks of 512 rows
  (8 w x 64 v). Per block:
    X^T[feat, row]   via PE transpose of DMA'd edge tiles
    gates^T          = Wg[i|g|o].T @ X^T          (f-gate skipped: c0 = 0)
    h^T              = sig(o)*tanh(sig(i)*tanh(g))     (ACT + DVE, bf16)
    w_edge           = sigmoid(W_lout.T @ h^T)    (bf16 matmul, M=1)
    msg^T            = Wmsg_bot.T @ X^T + P^T[w]  (P^T = Wmsg_top.T@node^T+b,
                                                   broadcast over v via 0-step AP)
    m[row, feat]     = msg^T_tile.T @ W_mrg       (layout flip: rows on partitions)
    LayerNorm        bn_stats/bn_aggr, 1/sqrt(var+eps), fused tensor_scalar
    GELU (erf)       ACT
    wm               = w_edge * gelu   (bf16)
    acc[v, feat]    += I2stack.T @ wm             (sum over w, psum-resident
                                                   across the whole kernel)
  out = node + acc.
"""
import numpy as np
import ml_dtypes
from contextlib import ExitStack

import concourse.mybir as mybir
import concourse.tile as tile
from concourse import bacc
from concourse.bass_utils import run_bass_kernel_spmd
from concourse.masks import make_identity

F32 = mybir.dt.float32
F32R = mybir.dt.float32r
BF16 = mybir.dt.bfloat16
AF = mybir.ActivationFunctionType
OP = mybir.AluOpType

B = 8           # batch == number of cores
N = 64          # nodes
D = 768         # feature dim
H = 256         # lstm hidden
ROWS = N * N    # 4096 edge rows per core
BLK = 512       # rows per block (8 w x 64 v)
NBLK = ROWS // BLK
TPB = BLK // 128
KD = D // 128
LN_EPS = 1e-12


def build(apply_bmrg=True, apply_lng=True, apply_lnb=True, reps=1, mmdt=F32R, v=None):
    v = {**dict(pst_bufs=1, psm_bufs=3, xt_bufs=2, ps1_bufs=2, copy_eng="mix",
                dmat=False, ecopy=False), **(v or {})}
    if v["dmat"]:
        assert mmdt == BF16
    nc = bacc.Bacc(None)

    edge = nc.dram_tensor("edge", (ROWS, D), F32, kind="ExternalInput")
    node = nc.dram_tensor("node", (N, D), F32, kind="ExternalInput")
    Wg = nc.dram_tensor("W_gates", (D, 4 * H), F32, kind="ExternalInput")
    bg = nc.dram_tensor("b_gates", (4 * H,), F32, kind="ExternalInput")
    Wl = nc.dram_tensor("W_lout", (H, 1), F32, kind="ExternalInput")
    bl = nc.dram_tensor("b_lout", (1,), F32, kind="ExternalInput")
    Wm = nc.dram_tensor("W_msg", (2 * D, D), F32, kind="ExternalInput")
    bm = nc.dram_tensor("b_msg", (D,), F32, kind="ExternalInput")
    Wr = nc.dram_tensor("W_mrg", (D, D), F32, kind="ExternalInput")
    br = nc.dram_tensor("b_mrg", (D,), F32, kind="ExternalInput")
    lg = nc.dram_tensor("ln_g", (D,), F32, kind="ExternalInput")
    lb = nc.dram_tensor("ln_b", (D,), F32, kind="ExternalInput")
    out = nc.dram_tensor("out", (N, D), F32, kind="ExternalOutput")

    # stacked identity [128, 64] bf16: partitions (w_local 2, v 64) -> v
    i2_np = np.tile(np.eye(N, dtype=np.float32), (2, 1)).astype(ml_dtypes.bfloat16)
    i2_dram = nc.inline_tensor(i2_np, name="i2_stack")
    onesrow_dram = nc.inline_tensor(np.ones((1, 128), np.float32), name="ones_row")

    with tile.TileContext(nc) as tc, ExitStack() as ctx:
        W = ctx.enter_context(tc.tile_pool(name="W", bufs=1))          # persistent
        xnp = ctx.enter_context(tc.tile_pool(name="xn", bufs=3))
        xtp = ctx.enter_context(tc.tile_pool(name="xt", bufs=v["xt_bufs"]))
        hp = ctx.enter_context(tc.tile_pool(name="h", bufs=2))
        msgp = ctx.enter_context(tc.tile_pool(name="msg", bufs=2))
        tmp = ctx.enter_context(tc.tile_pool(name="tmp", bufs=4))
        lnp = ctx.enter_context(tc.tile_pool(name="ln", bufs=4))
        wmp = ctx.enter_context(tc.tile_pool(name="wm", bufs=2))
        sml = ctx.enter_context(tc.tile_pool(name="sml", bufs=4))
        drp = ctx.enter_context(tc.tile_pool(name="dr", bufs=2, space="DRAM"))
        if v["dmat"]:
            e16p = ctx.enter_context(tc.tile_pool(name="e16", bufs=4, space="DRAM"))

        ps1 = ctx.enter_context(tc.tile_pool(name="ps1", bufs=v["ps1_bufs"], space="PSUM"))
        psm = ctx.enter_context(tc.tile_pool(name="psm", bufs=v["psm_bufs"], space="PSUM"))
        psf = ctx.enter_context(tc.tile_pool(name="psf", bufs=1, space="PSUM"))
        pst = ps1 if v["dmat"] else ctx.enter_context(
            tc.tile_pool(name="pst", bufs=v["pst_bufs"], space="PSUM"))

        # ---------------- persistent weights / constants ----------------
        ident = W.tile([128, 128], F32, tag="ident")
        make_identity(nc, ident[:])

        # W_gates cols: i=[0:256], g=[512:768], o=[768:1024] -> packed [i|g|o]
        wg_sb = W.tile([128, KD, D], mmdt, tag="wg")
        for k in range(KD):
            for j, (lo, hi) in enumerate([(0, 256), (512, 768), (768, 1024)]):
                nc.gpsimd.dma_start(wg_sb[:, k, j * 256:(j + 1) * 256],
                                    Wg[k * 128:(k + 1) * 128, lo:hi])
        wmt_sb = W.tile([128, KD, D], mmdt, tag="wmt")
        wmb_sb = W.tile([128, KD, D], mmdt, tag="wmb")
        wmg_sb = W.tile([128, KD, D], mmdt, tag="wmg")
        for k in range(KD):
            nc.gpsimd.dma_start(wmt_sb[:, k, :], Wm[k * 128:(k + 1) * 128, :])
            nc.gpsimd.dma_start(wmb_sb[:, k, :], Wm[D + k * 128:D + (k + 1) * 128, :])
            nc.gpsimd.dma_start(wmg_sb[:, k, :], Wr[k * 128:(k + 1) * 128, :])
        wl_sb = W.tile([128, 2, 1], BF16, tag="wl")
        nc.gpsimd.dma_start(wl_sb[:, 0, :], Wl[0:128, :])
        nc.gpsimd.dma_start(wl_sb[:, 1, :], Wl[128:256, :])

        # biases: b_gates [1024] -> [128, 8]; chunk cols i0=0 i1=1 g0=4 g1=5 o0=6 o1=7
        bg_sb = W.tile([128, 8], F32, tag="bg")
        nc.sync.dma_start(bg_sb[:], bg[:].rearrange("(c p) -> p c", p=128))
        bm_sb = W.tile([128, KD], F32, tag="bm")
        nc.sync.dma_start(bm_sb[:], bm[:].rearrange("(c p) -> p c", p=128))
        bl_sb = W.tile([128, 1], F32, tag="bl")
        nc.sync.dma_start(bl_sb[:], bl[:].partition_broadcast(128))

        gfull = W.tile([128, D], F32, tag="gfull")
        nc.sync.dma_start(gfull[:], lg[:].partition_broadcast(128))
        bfull = W.tile([128, D], F32, tag="bfull")
        nc.sync.dma_start(bfull[:], lb[:].partition_broadcast(128))

        i2_sb = W.tile([128, N], BF16, tag="i2")
        nc.sync.dma_start(i2_sb[:], i2_dram[:])
        onesrow_f = W.tile([1, 128], F32, tag="onesrowf")
        nc.sync.dma_start(onesrow_f[:], onesrow_dram[:])
        onesrow = W.tile([1, 128], mmdt, tag="onesrow")
        nc.vector.tensor_copy(onesrow[:], onesrow_f[:])
        brrow = W.tile([1, D], mmdt, tag="brrow")
        nc.gpsimd.dma_start(brrow[:], br[:].rearrange("(a c) -> a c", a=1))

        eps_sb = W.tile([128, 1], F32, tag="eps")
        nc.gpsimd.memset(eps_sb[:], LN_EPS)

        node_sb = W.tile([N, D], F32, tag="node")
        nc.sync.dma_start(node_sb[:], node[:])

        # node^T [128, KD, 64] f32r via PE transpose
        node_t = W.tile([128, KD, N], mmdt, tag="nodet")
        for k in range(KD):
            if v["dmat"]:
                ptt_full = pst.tile([128, 512], F32, tag="s1", name=f"ptn_{k}")
                ptt = ptt_full[:, 0:128]
            else:
                ptt = pst.tile([128, 128], F32, tag="tp", name=f"ptn_{k}")
            nc.tensor.transpose(ptt[:, 0:N], node_sb[0:N, k * 128:(k + 1) * 128],
                                ident[0:N, 0:N])
            nc.vector.tensor_copy(node_t[:, k, :], ptt[:, 0:N])

        # P^T [128, KD, 64] = Wmsg_top.T @ node^T (+ b_msg)
        p_sb = W.tile([128, KD, N], F32, tag="p")
        for m in range(KD):
            pp = ps1.tile([128, 512], F32, tag="s1")
            for k in range(KD):
                nc.tensor.matmul(pp[:, 0:N], wmt_sb[:, k, m * 128:(m + 1) * 128],
                                 node_t[:, k, :], start=(k == 0), stop=(k == KD - 1))
            nc.vector.tensor_scalar(p_sb[:, m, :], pp[:, 0:N], bm_sb[:, m:m + 1],
                                    None, OP.add)

        # final accumulators [64, 384] x2 (psum, live across main loop)
        acc_lo = psf.tile([N, 384], F32, tag="acclo")
        acc_hi = psf.tile([N, 384], F32, tag="acchi")

        out_sb = W.tile([N, D], F32, tag="out")

        # ---------------- main loop (body repeated `reps` times for timing) ----
        def body():
            for blk in range(NBLK):
                # 1. load (w-major: tile t covers w = blk*8+2t, +1) + 2. transpose
                e3 = edge[:].rearrange("(v w) d -> w v d", w=N)
                xt = xtp.tile([128, KD, BLK], mmdt, tag="xt")
                if v["dmat"]:
                    # per k: cast+reorder (w-major) into contiguous bf16 staging,
                    # then xbar-transpose into X^T. Strided-src transpose is
                    # broken on HW; contiguous staging is exact.
                    for k in range(KD):
                        ek = e16p.tile([BLK, 128], BF16, tag="ek", name=f"ek_{blk}_{k}")
                        nc.gpsimd.dma_start(
                            ek[:].rearrange("(w v) c -> w v c", w=8),
                            e3[blk * 8:(blk + 1) * 8][:, :, k * 128:(k + 1) * 128])
                        nc.sync.dma_start(xt[:, k, :], ek[:], transpose=True)
                else:
                    for t in range(TPB):
                        w0 = blk * 8 + 2 * t
                        xn = xnp.tile([128, D], F32, tag="xn")
                        nc.sync.dma_start(xn[0:N, :], e3[w0])
                        nc.sync.dma_start(xn[N:128, :], e3[w0 + 1])
                        for k in range(KD):
                            ptt = pst.tile([128, 128], F32, tag="tp")
                            nc.tensor.transpose(ptt[:], xn[:, k * 128:(k + 1) * 128],
                                                ident[:])
                            dst = xt[:, k, t * 128:(t + 1) * 128]
                            ce = v["copy_eng"]
                            if ce == "mix":
                                ce = "dve" if (t * KD + k) % 2 == 0 else "act"
                            if ce == "dve":
                                nc.vector.tensor_copy(dst, ptt[:])
                            else:
                                nc.scalar.activation(dst, ptt[:], AF.Identity)

                # 3. gates (order i,g,o per half; bias cols 0,4,1,5,6,7)
                def gate_mm(mchunk):
                    pg = ps1.tile([128, BLK], F32, tag="s1")
                    for k in range(KD):
                        nc.tensor.matmul(pg[:], wg_sb[:, k, mchunk * 128:(mchunk + 1) * 128],
                                         xt[:, k, :], start=(k == 0), stop=(k == KD - 1))
                    return pg

                h_sb = hp.tile([128, 2, BLK], BF16, tag="h")
                for half in range(2):
                    pg_i = gate_mm(half)
                    sig_i = tmp.tile([128, BLK], F32, tag="tmp")
                    nc.scalar.activation(sig_i[:], pg_i[:], AF.Sigmoid,
                                         bias=bg_sb[:, half:half + 1])
                    pg_g = gate_mm(2 + half)
                    tan_g = tmp.tile([128, BLK], F32, tag="tmp")
                    nc.scalar.activation(tan_g[:], pg_g[:], AF.Tanh,
                                         bias=bg_sb[:, 4 + half:5 + half])
                    c_t = tmp.tile([128, BLK], F32, tag="tmp")
                    nc.vector.tensor_tensor(c_t[:], sig_i[:], tan_g[:], OP.mult)
                    tan_c = tmp.tile([128, BLK], F32, tag="tmp")
                    nc.scalar.activation(tan_c[:], c_t[:], AF.Tanh)
                    pg_o = gate_mm(4 + half)
                    sig_o = tmp.tile([128, BLK], F32, tag="tmp")
                    nc.scalar.activation(sig_o[:], pg_o[:], AF.Sigmoid,
                                         bias=bg_sb[:, 6 + half:7 + half])
                    nc.vector.tensor_tensor(h_sb[:, half, :], sig_o[:], tan_c[:], OP.mult)

                # 4. edge weight -> wt [128 rows, TPB] via DRAM bounce reshape
                pw = ps1.tile([1, BLK], F32, tag="s1")
                for k in range(2):
                    nc.tensor.matmul(pw[:], wl_sb[:, k, :], h_sb[:, k, :],
                                     start=(k == 0), stop=(k == 1))
                wrow = sml.tile([1, BLK], F32, tag="wrow")
                nc.vector.tensor_copy(wrow[:], pw[:])
                wdr = drp.tile([1, BLK], F32, tag="wdr")
                nc.sync.dma_start(wdr[:], wrow[:])
                wt_pre = sml.tile([128, TPB], F32, tag="wtpre")
                nc.sync.dma_start(wt_pre[:],
                                  wdr[0:1, :].rearrange("a (t p) -> (a p) t", p=128))
                wt = sml.tile([128, TPB], F32, tag="wt")
                nc.scalar.activation(wt[:], wt_pre[:], AF.Sigmoid, bias=bl_sb[:])

                # 5. message  msg^T = Wmsg_bot.T @ X^T + P^T[w] (bcast over v)
                msg = msgp.tile([128, KD, BLK], mmdt, tag="msg")
                for m in range(KD):
                    pmb = ps1.tile([128, BLK], F32, tag="s1")
                    for k in range(KD):
                        nc.tensor.matmul(pmb[:], wmb_sb[:, k, m * 128:(m + 1) * 128],
                                         xt[:, k, :], start=(k == 0), stop=(k == KD - 1))
                    nc.vector.tensor_tensor(
                        msg[:, m, :].rearrange("p (w v) -> p w v", w=8),
                        pmb[:].rearrange("p (w v) -> p w v", w=8),
                        p_sb[:, m, blk * 8:blk * 8 + 8][:, :, None]
                            .broadcast_to((128, 8, N)),
                        OP.add)

                # 6-8. merge + LN + gelu + weighted reduce, per row-tile
                for t in range(TPB):
                    mlo = psm.tile([128, 384], F32, tag="pm")
                    mhi = psm.tile([128, 384], F32, tag="pm")
                    for k in range(KD):
                        lhs = msg[:, k, t * 128:(t + 1) * 128]
                        nc.tensor.matmul(mlo[:], lhs, wmg_sb[:, k, 0:384],
                                         start=(k == 0),
                                         stop=(k == KD - 1) and not apply_bmrg)
                        nc.tensor.matmul(mhi[:], lhs, wmg_sb[:, k, 384:768],
                                         start=(k == 0),
                                         stop=(k == KD - 1) and not apply_bmrg)
                    if apply_bmrg:
                        nc.tensor.matmul(mlo[:], onesrow[:], brrow[0:1, 0:384],
                                         start=False, stop=True)
                        nc.tensor.matmul(mhi[:], onesrow[:], brrow[0:1, 384:768],
                                         start=False, stop=True)

                    # copy psum out immediately so the merge psum slot frees
                    # after one op instead of after the whole LN chain
                    if v["ecopy"]:
                        ms = lnp.tile([128, 2, 384], F32, tag="ms")
                        nc.scalar.activation(ms[:, 0, :], mlo[:], AF.Identity)
                        nc.scalar.activation(ms[:, 1, :], mhi[:], AF.Identity)
                        src0, src1 = ms[:, 0, :], ms[:, 1, :]
                    else:
                        src0, src1 = mlo[:], mhi[:]
                    stats = sml.tile([128, 2, 6], F32, tag="stats")
                    nc.vector.bn_stats(stats[:, 0, :], src0)
                    nc.vector.bn_stats(stats[:, 1, :], src1)
                    mv = sml.tile([128, 2], F32, tag="mv")
                    nc.vector.bn_aggr(mv[:], stats[:])
                    sd = sml.tile([128, 1], F32, tag="sd")
                    nc.scalar.activation(sd[:], mv[:, 1:2], AF.Sqrt, bias=eps_sb[:])
                    istd = sml.tile([128, 1], F32, tag="istd")
                    nc.vector.reciprocal(istd[:], sd[:])

                    wm = wmp.tile([128, 2, 384], BF16, tag="wm")
                    for hf, mps in ((0, src0), (1, src1)):
                        y = lnp.tile([128, 384], F32, tag="y")
                        nc.vector.tensor_scalar(y[:], mps, mv[:, 0:1], istd[:],
                                                OP.subtract, OP.mult)
                        if apply_lng:
                            z = lnp.tile([128, 384], F32, tag="y")
                            nc.vector.tensor_tensor(
                                z[:], y[:], gfull[:, hf * 384:(hf + 1) * 384], OP.mult)
                            y = z
                        if apply_lnb:
                            z = lnp.tile([128, 384], F32, tag="y")
                            nc.vector.tensor_tensor(
                                z[:], y[:], bfull[:, hf * 384:(hf + 1) * 384], OP.add)
                            y = z
                        gl = lnp.tile([128, 384], F32, tag="y")
                        nc.scalar.activation(gl[:], y[:], AF.Gelu)
                        nc.vector.tensor_scalar(wm[:, hf, :], gl[:], wt[:, t:t + 1],
                                                None, OP.mult)

                    first = blk == 0 and t == 0
                    last = blk == NBLK - 1 and t == TPB - 1
                    nc.tensor.matmul(acc_lo[:], i2_sb[:], wm[:, 0, :],
                                     start=first, stop=last, skip_group_check=True)
                    nc.tensor.matmul(acc_hi[:], i2_sb[:], wm[:, 1, :],
                                     start=first, stop=last, skip_group_check=True)

            # 9. residual + store
            nc.vector.scalar_tensor_tensor(out_sb[:, 0:384], acc_lo[:], 0.0,
                                           node_sb[:, 0:384], OP.add, OP.add)
            nc.vector.scalar_tensor_tensor(out_sb[:, 384:768], acc_hi[:], 0.0,
                                           node_sb[:, 384:768], OP.add, OP.add)
            nc.sync.dma_start(out[:], out_sb[:])

        if reps == 1:
            body()
        else:
            with tc.For_i(0, reps, 1):
                body()

    nc.finalize()
    return nc


_CACHE = {}


MMDT = F32R
VOPT = None


def _get_nc(flags, reps=1):
    key = (flags, reps, MMDT, repr(VOPT))
    if key not in _CACHE:
        _CACHE[key] = build(apply_bmrg=flags[0], apply_lng=flags[1],
                            apply_lnb=flags[2], reps=reps, mmdt=MMDT, v=VOPT)
    return _CACHE[key]


def _flags(inputs):
    return (bool(np.any(inputs["b_mrg"])),
            not bool(np.allclose(inputs["ln_g"], 1.0)),
            bool(np.any(inputs["ln_b"])))


def _in_maps(inputs):
    e = np.ascontiguousarray(inputs["edge_features"], np.float32).reshape(B, ROWS, D)
    nf = np.ascontiguousarray(inputs["node_features"], np.float32)
    wkeys = ["W_gates", "b_gates", "W_lout", "b_lout", "W_msg", "b_msg",
             "W_mrg", "b_mrg", "ln_g", "ln_b"]
    w = {k: np.ascontiguousarray(inputs[k], np.float32) for k in wkeys}
    return [dict(edge=e[b], node=nf[b], **w) for b in range(B)]


def kernel(**inputs):
    nc = _get_nc(_flags(inputs))
    res = run_bass_kernel_spmd(nc, _in_maps(inputs), list(range(B)))
    return np.stack([res.results[b]["out"] for b in range(B)]).astype(np.float32)


def run_timed(inputs, reps):
    """Run the reps-looped variant once; returns (output, wall_seconds)."""
    import time
    nc = _get_nc(_flags(inputs), reps=reps)
    maps = _in_maps(inputs)
    t0 = time.time()
    res = run_bass_kernel_spmd(nc, maps, list(range(B)))
    dt = time.time() - t0
    out = np.stack([res.results[b]["out"] for b in range(B)]).astype(np.float32)
    return out, dt


# revision 17
# speedup vs baseline: 3.0978x; 1.0839x over previous
"""GPNNCell (gnn_message_passing) Trainium2 Bass kernel.

Full-input contract: kernel(**inputs) takes the complete tensors from
setup_inputs() and returns the full [8, 64, 64->sum, 768] output, i.e.
node_features + sum_w weight_edge * merged_message   -> [8, 64, 768].

Distribution: data-parallel over batch B=8, one batch element per NeuronCore,
no collectives. Per core the whole cell is computed as a chain of f32r
(TF32-like, 1 cyc/row) matmuls on the tensor engine:

  edge rows are processed source-node(w)-major in 8 blocks of 512 rows
  (8 w x 64 v). Per block:
    X^T[feat, row]   via PE transpose of DMA'd edge tiles
    gates^T          = Wg[i|g|o].T @ X^T          (f-gate skipped: c0 = 0)
    h^T              = sig(o)*tanh(sig(i)*tanh(g))     (ACT + DVE, bf16)
    w_edge           = sigmoid(W_lout.T @ h^T)    (bf16 matmul, M=1)
    msg^T            = Wmsg_bot.T @ X^T + P^T[w]  (P^T = Wmsg_top.T@node^T+b,
                                                   broadcast over v via 0-step AP)
    m[row, feat]     = msg^T_tile.T @ W_mrg       (layout flip: rows on partitions)
    LayerNorm        bn_stats/bn_aggr, 1/sqrt(var+eps), fused tensor_scalar
    GELU (erf)       ACT
    wm               = w_edge * gelu   (bf16)
    acc[v, feat]    += I2stack.T @ wm             (sum over w, psum-resident
                                                   across the whole kernel)
  out = node + acc.
"""
import numpy as np
import ml_dtypes
from contextlib import ExitStack

import concourse.mybir as mybir
import concourse.tile as tile
from concourse import bacc
from concourse.bass_utils import run_bass_kernel_spmd
from concourse.masks import make_identity

F32 = mybir.dt.float32
F32R = mybir.dt.float32r
BF16 = mybir.dt.bfloat16
AF = mybir.ActivationFunctionType
OP = mybir.AluOpType

B = 8           # batch == number of cores
N = 64          # nodes
D = 768         # feature dim
H = 256         # lstm hidden
ROWS = N * N    # 4096 edge rows per core
BLK = 512       # rows per block (8 w x 64 v)
NBLK = ROWS // BLK
TPB = BLK // 128
KD = D // 128
LN_EPS = 1e-12


def build(apply_bmrg=True, apply_lng=True, apply_lnb=True, reps=1, mmdt=F32R, v=None):
    v = {**dict(pst_bufs=2, psm_bufs=2, xt_bufs=2, ps1_bufs=3, copy_eng="mix",
                dmat=False, ecopy=False), **(v or {})}
    if v["dmat"]:
        assert mmdt == BF16
    nc = bacc.Bacc(None)

    edge = nc.dram_tensor("edge", (ROWS, D), F32, kind="ExternalInput")
    node = nc.dram_tensor("node", (N, D), F32, kind="ExternalInput")
    Wg = nc.dram_tensor("W_gates", (D, 4 * H), F32, kind="ExternalInput")
    bg = nc.dram_tensor("b_gates", (4 * H,), F32, kind="ExternalInput")
    Wl = nc.dram_tensor("W_lout", (H, 1), F32, kind="ExternalInput")
    bl = nc.dram_tensor("b_lout", (1,), F32, kind="ExternalInput")
    Wm = nc.dram_tensor("W_msg", (2 * D, D), F32, kind="ExternalInput")
    bm = nc.dram_tensor("b_msg", (D,), F32, kind="ExternalInput")
    Wr = nc.dram_tensor("W_mrg", (D, D), F32, kind="ExternalInput")
    br = nc.dram_tensor("b_mrg", (D,), F32, kind="ExternalInput")
    lg = nc.dram_tensor("ln_g", (D,), F32, kind="ExternalInput")
    lb = nc.dram_tensor("ln_b", (D,), F32, kind="ExternalInput")
    out = nc.dram_tensor("out", (N, D), F32, kind="ExternalOutput")

    # stacked identity [128, 64] bf16: partitions (w_local 2, v 64) -> v
    i2_np = np.tile(np.eye(N, dtype=np.float32), (2, 1)).astype(ml_dtypes.bfloat16)
    i2_dram = nc.inline_tensor(i2_np, name="i2_stack")
    onesrow_dram = nc.inline_tensor(np.ones((1, 128), np.float32), name="ones_row")

    with tile.TileContext(nc) as tc, ExitStack() as ctx:
        W = ctx.enter_context(tc.tile_pool(name="W", bufs=1))          # persistent
        xnp = ctx.enter_context(tc.tile_pool(name="xn", bufs=5))
        xtp = ctx.enter_context(tc.tile_pool(name="xt", bufs=v["xt_bufs"]))
        hp = ctx.enter_context(tc.tile_pool(name="h", bufs=2))
        msgp = ctx.enter_context(tc.tile_pool(name="msg", bufs=2))
        tmp = ctx.enter_context(tc.tile_pool(name="tmp", bufs=4))
        lnp = ctx.enter_context(tc.tile_pool(name="ln", bufs=5))
        wmp = ctx.enter_context(tc.tile_pool(name="wm", bufs=2))
        sml = ctx.enter_context(tc.tile_pool(name="sml", bufs=6))
        drp = ctx.enter_context(tc.tile_pool(name="dr", bufs=2, space="DRAM"))
        if v["dmat"]:
            e16p = ctx.enter_context(tc.tile_pool(name="e16", bufs=4, space="DRAM"))

        ps1 = ctx.enter_context(tc.tile_pool(name="ps1", bufs=v["ps1_bufs"], space="PSUM"))
        psm = ctx.enter_context(tc.tile_pool(name="psm", bufs=v["psm_bufs"], space="PSUM"))
        psf = ctx.enter_context(tc.tile_pool(name="psf", bufs=1, space="PSUM"))
        pst = ps1 if v["dmat"] else ctx.enter_context(
            tc.tile_pool(name="pst", bufs=v["pst_bufs"], space="PSUM"))

        # ---------------- persistent weights / constants ----------------
        ident = W.tile([128, 128], F32, tag="ident")
        make_identity(nc, ident[:])

        # W_gates cols: i=[0:256], g=[512:768], o=[768:1024] -> packed [i|g|o]
        wg_sb = W.tile([128, KD, D], mmdt, tag="wg")
        for k in range(KD):
            for j, (lo, hi) in enumerate([(0, 256), (512, 768), (768, 1024)]):
                nc.gpsimd.dma_start(wg_sb[:, k, j * 256:(j + 1) * 256],
                                    Wg[k * 128:(k + 1) * 128, lo:hi])
        wmt_sb = W.tile([128, KD, D], mmdt, tag="wmt")
        wmb_sb = W.tile([128, KD, D], mmdt, tag="wmb")
        wmg_sb = W.tile([128, KD, D], mmdt, tag="wmg")
        for k in range(KD):
            nc.gpsimd.dma_start(wmt_sb[:, k, :], Wm[k * 128:(k + 1) * 128, :])
            nc.gpsimd.dma_start(wmb_sb[:, k, :], Wm[D + k * 128:D + (k + 1) * 128, :])
            nc.gpsimd.dma_start(wmg_sb[:, k, :], Wr[k * 128:(k + 1) * 128, :])
        # W_lout/2: compensates h being stored as 2*h = (tanh(o/2)+1)*tanh(c)
        wl_f = W.tile([128, 2, 1], F32, tag="wlf")
        nc.sync.dma_start(wl_f[:, 0, :], Wl[0:128, :])
        nc.sync.dma_start(wl_f[:, 1, :], Wl[128:256, :])
        wl_sb = W.tile([128, 2, 1], BF16, tag="wl")
        nc.vector.tensor_scalar(wl_sb[:, :, :], wl_f[:, :, :], 0.5, None, OP.mult)

        # biases: b_gates [1024] -> [128, 8]; chunk cols i0=0 i1=1 g0=4 g1=5 o0=6 o1=7
        bg_sb = W.tile([128, 8], F32, tag="bg")
        nc.sync.dma_start(bg_sb[:], bg[:].rearrange("(c p) -> p c", p=128))
        bm_sb = W.tile([128, KD], F32, tag="bm")
        nc.sync.dma_start(bm_sb[:], bm[:].rearrange("(c p) -> p c", p=128))
        bl_sb = W.tile([128, 1], F32, tag="bl")
        nc.sync.dma_start(bl_sb[:], bl[:].partition_broadcast(128))
        # halved biases for the sigmoid->tanh rewrite: sig(x)=0.5*tanh(x/2)+0.5
        bg2_sb = W.tile([128, 8], F32, tag="bg2")
        nc.vector.tensor_scalar(bg2_sb[:], bg_sb[:], 0.5, None, OP.mult)
        bl2_sb = W.tile([128, 1], F32, tag="bl2")
        nc.vector.tensor_scalar(bl2_sb[:], bl_sb[:], 0.5, None, OP.mult)

        gfull = W.tile([128, D], F32, tag="gfull")
        nc.sync.dma_start(gfull[:], lg[:].partition_broadcast(128))
        bfull = W.tile([128, D], F32, tag="bfull")
        nc.sync.dma_start(bfull[:], lb[:].partition_broadcast(128))

        i2_sb = W.tile([128, N], BF16, tag="i2")
        nc.sync.dma_start(i2_sb[:], i2_dram[:])
        onesrow_f = W.tile([1, 128], F32, tag="onesrowf")
        nc.sync.dma_start(onesrow_f[:], onesrow_dram[:])
        onesrow = W.tile([1, 128], mmdt, tag="onesrow")
        nc.vector.tensor_copy(onesrow[:], onesrow_f[:])
        brrow = W.tile([1, D], mmdt, tag="brrow")
        nc.gpsimd.dma_start(brrow[:], br[:].rearrange("(a c) -> a c", a=1))

        eps_sb = W.tile([128, 1], F32, tag="eps")
        nc.gpsimd.memset(eps_sb[:], LN_EPS)

        node_sb = W.tile([N, D], F32, tag="node")
        nc.sync.dma_start(node_sb[:], node[:])

        # node^T [128, KD, 64] f32r via PE transpose
        node_t = W.tile([128, KD, N], mmdt, tag="nodet")
        for k in range(KD):
            if v["dmat"]:
                ptt_full = pst.tile([128, 512], F32, tag="s1", name=f"ptn_{k}")
                ptt = ptt_full[:, 0:128]
            else:
                ptt = pst.tile([128, 128], F32, tag="tp", name=f"ptn_{k}")
            nc.tensor.transpose(ptt[:, 0:N], node_sb[0:N, k * 128:(k + 1) * 128],
                                ident[0:N, 0:N])
            nc.vector.tensor_copy(node_t[:, k, :], ptt[:, 0:N])

        # P^T [128, KD, 64] = Wmsg_top.T @ node^T (+ b_msg)
        p_sb = W.tile([128, KD, N], F32, tag="p")
        for m in range(KD):
            pp = ps1.tile([128, 512], F32, tag="s1")
            for k in range(KD):
                nc.tensor.matmul(pp[:, 0:N], wmt_sb[:, k, m * 128:(m + 1) * 128],
                                 node_t[:, k, :], start=(k == 0), stop=(k == KD - 1))
            nc.vector.tensor_scalar(p_sb[:, m, :], pp[:, 0:N], bm_sb[:, m:m + 1],
                                    None, OP.add)

        # final accumulator, one bank: partitions 0:64 = lo half, 64:128 = hi
        acc = psf.tile([128, 384], F32, tag="acc")
        acc_lo = acc[0:N, :]
        acc_hi = acc[N:128, :]

        out_sb = W.tile([N, D], F32, tag="out")

        # ---------------- main loop (body repeated `reps` times for timing) ----
        def body():
            for blk in range(NBLK):
                # 1. load (w-major: tile t covers w = blk*8+2t, +1) + 2. transpose
                e3 = edge[:].rearrange("(v w) d -> w v d", w=N)
                xt = xtp.tile([128, KD, BLK], mmdt, tag="xt")
                if v["dmat"]:
                    # per k: cast+reorder (w-major) into contiguous bf16 staging,
                    # then xbar-transpose into X^T. Strided-src transpose is
                    # broken on HW; contiguous staging is exact.
                    for k in range(KD):
                        ek = e16p.tile([BLK, 128], BF16, tag="ek", name=f"ek_{blk}_{k}")
                        nc.gpsimd.dma_start(
                            ek[:].rearrange("(w v) c -> w v c", w=8),
                            e3[blk * 8:(blk + 1) * 8][:, :, k * 128:(k + 1) * 128])
                        nc.sync.dma_start(xt[:, k, :], ek[:], transpose=True)
                else:
                    for t in range(TPB):
                        w0 = blk * 8 + 2 * t
                        xn = xnp.tile([128, D], F32, tag="xn")
                        nc.sync.dma_start(xn[0:N, :], e3[w0])
                        nc.sync.dma_start(xn[N:128, :], e3[w0 + 1])
                        for k in range(KD):
                            ptt = pst.tile([128, 128], F32, tag="tp")
                            nc.tensor.transpose(ptt[:], xn[:, k * 128:(k + 1) * 128],
                                                ident[:])
                            dst = xt[:, k, t * 128:(t + 1) * 128]
                            ce = v["copy_eng"]
                            if ce == "mix":
                                ce = "dve" if (t * KD + k) % 2 == 0 else "act"
                            if ce == "dve":
                                nc.vector.tensor_copy(dst, ptt[:])
                            else:
                                nc.scalar.activation(dst, ptt[:], AF.Identity)

                # 3. gates (order i,g,o per half; bias cols 0,4,1,5,6,7)
                def gate_mm(mchunk):
                    pg = ps1.tile([128, BLK], F32, tag="s1")
                    for k in range(KD):
                        nc.tensor.matmul(pg[:], wg_sb[:, k, mchunk * 128:(mchunk + 1) * 128],
                                         xt[:, k, :], start=(k == 0), stop=(k == KD - 1))
                    return pg

                # all-tanh gates (sigmoid-free => one ACT table set):
                #   sig(x) = 0.5*tanh(x/2) + 0.5
                #   c  = sig(i)*tanh(g); tanh(c) = tanh(0.5*(tanh(i/2)+1)*tanh(g))
                #   h2 = (tanh(o/2)+1)*tanh(c) = 2*h, compensated in W_lout/2
                h_sb = hp.tile([128, 2, BLK], BF16, tag="h")
                for half in range(2):
                    pg_i = gate_mm(half)
                    tan_i = tmp.tile([128, BLK], F32, tag="tmp")
                    nc.scalar.activation(tan_i[:], pg_i[:], AF.Tanh, scale=0.5,
                                         bias=bg2_sb[:, half:half + 1])
                    pg_g = gate_mm(2 + half)
                    tan_g = tmp.tile([128, BLK], F32, tag="tmp")
                    nc.scalar.activation(tan_g[:], pg_g[:], AF.Tanh,
                                         bias=bg_sb[:, 4 + half:5 + half])
                    c_t = tmp.tile([128, BLK], F32, tag="tmp")
                    nc.vector.scalar_tensor_tensor(c_t[:], tan_i[:], 1.0, tan_g[:],
                                                   OP.add, OP.mult)
                    tan_c = tmp.tile([128, BLK], F32, tag="tmp")
                    nc.scalar.activation(tan_c[:], c_t[:], AF.Tanh, scale=0.5)
                    pg_o = gate_mm(4 + half)
                    tan_o = tmp.tile([128, BLK], F32, tag="tmp")
                    nc.scalar.activation(tan_o[:], pg_o[:], AF.Tanh, scale=0.5,
                                         bias=bg2_sb[:, 6 + half:7 + half])
                    nc.vector.scalar_tensor_tensor(h_sb[:, half, :], tan_o[:], 1.0,
                                                   tan_c[:], OP.add, OP.mult)

                # 4. edge weight -> wt [128 rows, TPB] via DRAM bounce reshape
                pw = ps1.tile([1, BLK], F32, tag="s1")
                for k in range(2):
                    nc.tensor.matmul(pw[:], wl_sb[:, k, :], h_sb[:, k, :],
                                     start=(k == 0), stop=(k == 1))
                wrow = sml.tile([1, BLK], F32, tag="wrow")
                nc.vector.tensor_copy(wrow[:], pw[:])
                wdr = drp.tile([1, BLK], F32, tag="wdr")
                nc.sync.dma_start(wdr[:], wrow[:])
                wt_pre = sml.tile([128, TPB], F32, tag="wtpre")
                nc.sync.dma_start(wt_pre[:],
                                  wdr[0:1, :].rearrange("a (t p) -> (a p) t", p=128))
                wt_t = sml.tile([128, TPB], F32, tag="wtt")
                nc.scalar.activation(wt_t[:], wt_pre[:], AF.Tanh, scale=0.5,
                                     bias=bl2_sb[:])
                wt = sml.tile([128, TPB], F32, tag="wt")
                nc.vector.tensor_scalar(wt[:], wt_t[:], 0.5, 0.5, OP.mult, OP.add)

                # 5. message  msg^T = Wmsg_bot.T @ X^T + P^T[w] (bcast over v)
                msg = msgp.tile([128, KD, BLK], mmdt, tag="msg")
                for m in range(KD):
                    pmb = ps1.tile([128, BLK], F32, tag="s1")
                    for k in range(KD):
                        nc.tensor.matmul(pmb[:], wmb_sb[:, k, m * 128:(m + 1) * 128],
                                         xt[:, k, :], start=(k == 0), stop=(k == KD - 1))
                    nc.vector.tensor_tensor(
                        msg[:, m, :].rearrange("p (w v) -> p w v", w=8),
                        pmb[:].rearrange("p (w v) -> p w v", w=8),
                        p_sb[:, m, blk * 8:blk * 8 + 8][:, :, None]
                            .broadcast_to((128, 8, N)),
                        OP.add)

                # 6-8. merge + LN + gelu + weighted reduce.
                # Two-phase: per-tile stats first (psum freed via ACT Identity
                # copies -- same table set as Gelu), then ONE batched Sqrt per
                # block so the ACT table only swaps gelu-set <-> sqrt-set twice
                # per block instead of twice per row-tile.
                mss = []
                mvl = []
                varb = sml.tile([128, TPB], F32, tag="varb")
                for t in range(TPB):
                    mlo = psm.tile([128, 384], F32, tag="pm")
                    mhi = psm.tile([128, 384], F32, tag="pm")
                    for k in range(KD):
                        lhs = msg[:, k, t * 128:(t + 1) * 128]
                        nc.tensor.matmul(mlo[:], lhs, wmg_sb[:, k, 0:384],
                                         start=(k == 0),
                                         stop=(k == KD - 1) and not apply_bmrg)
                        nc.tensor.matmul(mhi[:], lhs, wmg_sb[:, k, 384:768],
                                         start=(k == 0),
                                         stop=(k == KD - 1) and not apply_bmrg)
                    if apply_bmrg:
                        nc.tensor.matmul(mlo[:], onesrow[:], brrow[0:1, 0:384],
                                         start=False, stop=True)
                        nc.tensor.matmul(mhi[:], onesrow[:], brrow[0:1, 384:768],
                                         start=False, stop=True)
                    ms = lnp.tile([128, 2, 384], F32, tag="ms", name=f"ms_{blk}_{t}")
                    nc.scalar.activation(ms[:, 0, :], mlo[:], AF.Identity)
                    nc.scalar.activation(ms[:, 1, :], mhi[:], AF.Identity)
                    mss.append(ms)
                    stats = sml.tile([128, 2, 6], F32, tag="stats")
                    nc.vector.bn_stats(stats[:, 0, :], ms[:, 0, :])
                    nc.vector.bn_stats(stats[:, 1, :], ms[:, 1, :])
                    mv = sml.tile([128, 2], F32, tag="mv", name=f"mv_{blk}_{t}")
                    nc.vector.bn_aggr(mv[:], stats[:])
                    nc.vector.tensor_copy(varb[:, t:t + 1], mv[:, 1:2])
                    mvl.append(mv)

                sd = sml.tile([128, TPB], F32, tag="sd")
                nc.scalar.activation(sd[:], varb[:], AF.Sqrt, bias=eps_sb[:])
                istd = sml.tile([128, TPB], F32, tag="istd")
                nc.vector.reciprocal(istd[:], sd[:])

                for t in range(TPB):
                    ms = mss[t]
                    wm = wmp.tile([128, 2, 384], BF16, tag="wm")
                    for hf in range(2):
                        y = lnp.tile([128, 384], F32, tag="y")
                        nc.vector.tensor_scalar(y[:], ms[:, hf, :], mvl[t][:, 0:1],
                                                istd[:, t:t + 1], OP.subtract, OP.mult)
                        if apply_lng:
                            z = lnp.tile([128, 384], F32, tag="y")
                            nc.vector.tensor_tensor(
                                z[:], y[:], gfull[:, hf * 384:(hf + 1) * 384], OP.mult)
                            y = z
                        if apply_lnb:
                            z = lnp.tile([128, 384], F32, tag="y")
                            nc.vector.tensor_tensor(
                                z[:], y[:], bfull[:, hf * 384:(hf + 1) * 384], OP.add)
                            y = z
                        gl = lnp.tile([128, 384], F32, tag="y")
                        nc.scalar.activation(gl[:], y[:], AF.Gelu)
                        nc.vector.tensor_scalar(wm[:, hf, :], gl[:], wt[:, t:t + 1],
                                                None, OP.mult)

                    first = blk == 0 and t == 0
                    last = blk == NBLK - 1 and t == TPB - 1
                    nc.tensor.matmul(acc_lo, i2_sb[:], wm[:, 0, :],
                                     start=first, stop=last, skip_group_check=True)
                    nc.tensor.matmul(acc_hi, i2_sb[:], wm[:, 1, :],
                                     start=first, stop=last, skip_group_check=True)

            # 9. residual + store
            nc.vector.scalar_tensor_tensor(out_sb[:, 0:384], acc_lo, 0.0,
                                           node_sb[:, 0:384], OP.add, OP.add)
            nc.vector.scalar_tensor_tensor(out_sb[:, 384:768], acc_hi, 0.0,
                                           node_sb[:, 384:768], OP.add, OP.add)
            nc.sync.dma_start(out[:], out_sb[:])

        if reps == 1:
            body()
        else:
            with tc.For_i(0, reps, 1):
                body()

    nc.finalize()
    return nc


_CACHE = {}


MMDT = F32R
VOPT = None


def _get_nc(flags, reps=1):
    key = (flags, reps, MMDT, repr(VOPT))
    if key not in _CACHE:
        _CACHE[key] = build(apply_bmrg=flags[0], apply_lng=flags[1],
                            apply_lnb=flags[2], reps=reps, mmdt=MMDT, v=VOPT)
    return _CACHE[key]


def _flags(inputs):
    return (bool(np.any(inputs["b_mrg"])),
            not bool(np.allclose(inputs["ln_g"], 1.0)),
            bool(np.any(inputs["ln_b"])))


def _in_maps(inputs):
    e = np.ascontiguousarray(inputs["edge_features"], np.float32).reshape(B, ROWS, D)
    nf = np.ascontiguousarray(inputs["node_features"], np.float32)
    wkeys = ["W_gates", "b_gates", "W_lout", "b_lout", "W_msg", "b_msg",
             "W_mrg", "b_mrg", "ln_g", "ln_b"]
    w = {k: np.ascontiguousarray(inputs[k], np.float32) for k in wkeys}
    return [dict(edge=e[b], node=nf[b], **w) for b in range(B)]


def kernel(**inputs):
    nc = _get_nc(_flags(inputs))
    res = run_bass_kernel_spmd(nc, _in_maps(inputs), list(range(B)))
    return np.stack([res.results[b]["out"] for b in range(B)]).astype(np.float32)


def run_timed(inputs, reps):
    """Run the reps-looped variant once; returns (output, wall_seconds)."""
    import time
    nc = _get_nc(_flags(inputs), reps=reps)
    maps = _in_maps(inputs)
    t0 = time.time()
    res = run_bass_kernel_spmd(nc, maps, list(range(B)))
    dt = time.time() - t0
    out = np.stack([res.results[b]["out"] for b in range(B)]).astype(np.float32)
    return out, dt
